# revision 1
# baseline (speedup 1.0000x reference)
"""MoLoRA Trainium2 Bass kernel — r12: TT=512 compute tiles.

Same DMA shape as r8 (1.28 MB per stream per tile, x on SP HWDGE, store on
ACT HWDGE, base-accum on gpsimd SWDGE; delta lag-1, accum lag-2, store
lag-3) but the compute tile is 512 tokens (4 partition-halves), cutting
DVE/ACT instruction count ~30% and PE instruction count ~25% at identical
element throughput.  PSUM fits 8 banks via per-h-chunk transpose staging
tiles ([128, 4, 128] = 1 bank) and the shared h/lg router tag.
"""

import numpy as np
from contextlib import ExitStack

import concourse.bass as bass
import concourse.tile as tile
from concourse import bacc
from concourse import mybir
from concourse.bass import ts
from concourse.masks import make_identity
from concourse.bass_utils import run_bass_kernel_spmd

F32 = mybir.dt.float32
F32R = mybir.dt.float32r
AF = mybir.ActivationFunctionType
ALU = mybir.AluOpType
AX = mybir.AxisListType

H = 640
E = 5
R = 8
ER = E * R
RH = 256
HC = H // 128
RC = RH // 128
SCALING = 16.0 / R
N_CORES = 8
T_CORE = 4096
TT = 512          # compute tile (4 halves of 128 tokens)
JT = TT // 128    # 4


def build_kernel(t_core=T_CORE, niter=1, timing_mode=False, passes=1):
    assert t_core % TT == 0
    ntiles = t_core // TT
    nc = bacc.Bacc()

    if timing_mode:
        x_d = nc.dram_tensor("x_int", [t_core, H], F32)[:, :]
        base_d = nc.dram_tensor("base_int", [t_core, H], F32)[:, :]
        out_d = nc.dram_tensor("out_int", [t_core, H], F32)[:, :]
        dummy_d = nc.declare_dram_parameter("dummy_out", [1, 4], F32, isOutput=True)
    else:
        x_d = nc.declare_dram_parameter("x", [t_core, H], F32, isOutput=False)
        base_d = nc.declare_dram_parameter("base", [t_core, H], F32, isOutput=False)
        out_d = nc.declare_dram_parameter("out", [t_core, H], F32, isOutput=True)
        dummy_d = None
    w1_d = nc.declare_dram_parameter("W1", [H, RH], F32, isOutput=False)
    b1_d = nc.declare_dram_parameter("b1", [RH], F32, isOutput=False)
    w2_d = nc.declare_dram_parameter("W2", [RH, E], F32, isOutput=False)
    b2_d = nc.declare_dram_parameter("b2", [E], F32, isOutput=False)
    a_d = nc.declare_dram_parameter("A", [E, H, R], F32, isOutput=False)
    bm_d = nc.declare_dram_parameter("Bm", [E, R, H], F32, isOutput=False)

    with ExitStack() as ctx:
        tc = ctx.enter_context(tile.TileContext(nc))
        const = ctx.enter_context(tc.tile_pool(name="const", bufs=1))
        xin_p = ctx.enter_context(tc.tile_pool(name="xin", bufs=3))
        bout_p = ctx.enter_context(tc.tile_pool(name="bout", bufs=5))
        xt_p = ctx.enter_context(tc.tile_pool(name="xt", bufs=2))
        ht_p = ctx.enter_context(tc.tile_pool(name="ht", bufs=2))
        small_p = ctx.enter_context(tc.tile_pool(name="small", bufs=4))
        lw_p = ctx.enter_context(tc.tile_pool(name="lw", bufs=3))
        ps_xt = ctx.enter_context(tc.tile_pool(name="ps_xt", bufs=1, space="PSUM"))
        ps_rt = ctx.enter_context(tc.tile_pool(name="ps_rt", bufs=1, space="PSUM"))
        ps_low = ctx.enter_context(tc.tile_pool(name="ps_low", bufs=2, space="PSUM"))
        ps_wrt = ctx.enter_context(tc.tile_pool(name="ps_wrt", bufs=1, space="PSUM"))
        ps_dl = ctx.enter_context(tc.tile_pool(name="ps_dl", bufs=1, space="PSUM"))

        ident = const.tile([128, 128], F32)
        make_identity(nc, ident)
        ident_r = const.tile([128, 128], F32R)
        nc.vector.tensor_copy(out=ident_r, in_=ident)

        w1_sb = const.tile([128, HC, RH], F32R)
        nc.gpsimd.dma_start(
            out=w1_sb, in_=w1_d.bitcast(F32R).rearrange("(c p) m -> p c m", p=128)
        )
        b1_sb = const.tile([128, RC], F32)
        nc.gpsimd.dma_start(out=b1_sb, in_=b1_d.rearrange("(c p) -> p c", p=128))
        w2_sb = const.tile([128, RC, E], F32)
        nc.gpsimd.dma_start(out=w2_sb, in_=w2_d.rearrange("(c p) e -> p c e", p=128))
        b2_sb = const.tile([1, E], F32)
        nc.gpsimd.dma_start(out=b2_sb, in_=b2_d[:].unsqueeze(0))
        ones_sb = const.tile([1, 128], F32)
        nc.vector.memset(ones_sb, 1.0)
        acat_sb = const.tile([128, HC, E, R], F32R)
        for e in range(E):
            for c in range(HC):
                nc.gpsimd.dma_start(
                    out=acat_sb[:, c, e, :],
                    in_=a_d.bitcast(F32R)[e, c * 128 : (c + 1) * 128, :],
                )
        bcat_sb = const.tile([ER, H], F32R)
        for e in range(E):
            nc.gpsimd.dma_start(
                out=bcat_sb[e * R : (e + 1) * R, :], in_=bm_d.bitcast(F32R)[e, :, :]
            )

        if dummy_d is not None:
            dnm = const.tile([1, 4], F32)
            nc.vector.memset(dnm, 1.0)
            nc.sync.dma_start(out=dummy_d[:, :], in_=dnm)

        loop_ctx = tc.For_i(0, niter, 1) if niter > 1 else None
        if loop_ctx is not None:
            ctx.enter_context(loop_ctx)

        def emit_front(i):
            """1.28 MB x load + xT transposes staged per h-chunk"""
            tok = i * TT
            x_nat = xin_p.tile([128, JT, H], F32R)
            nc.sync.dma_start(
                out=x_nat,
                in_=x_d.bitcast(F32R)[tok : tok + TT, :].rearrange(
                    "(q p) h -> p q h", p=128
                ),
            )
            xt_sb = xt_p.tile([128, HC, TT], F32R)
            for c in range(HC):
                xtc = ps_xt.tile([128, JT, 128], F32R, tag="xtp")
                for q in range(JT):
                    nc.tensor.transpose(
                        out=xtc[:, q, :],
                        in_=x_nat[:, q, ts(c, 128)],
                        identity=ident_r,
                    )
                nc.any.tensor_copy(
                    out=xt_sb[:, c, :].rearrange("p (q t) -> p q t", q=JT), in_=xtc
                )
            return {"xt_sb": xt_sb, "tok": tok}

        def emit_router(st):
            xt_r = st["xt_sb"]
            h_ps = ps_rt.tile([128, RC, TT], F32, tag="rt")
            for c2 in range(RC):
                for c in range(HC):
                    nc.tensor.matmul(
                        out=h_ps[:, c2, :],
                        lhsT=w1_sb[:, c, ts(c2, 128)],
                        rhs=xt_r[:, c, :],
                        start=(c == 0),
                        stop=(c == HC - 1),
                    )
            ht_sb = ht_p.tile([128, RC, TT], F32)
            sg_sb = ht_p.tile([128, RC, TT], F32, tag="sg")
            for c2 in range(RC):
                nc.scalar.activation(
                    out=sg_sb[:, c2, :],
                    in_=h_ps[:, c2, :],
                    func=AF.Sigmoid,
                    bias=b1_sb[:, c2 : c2 + 1],
                )
                nc.vector.scalar_tensor_tensor(
                    out=ht_sb[:, c2, :],
                    in0=h_ps[:, c2, :],
                    scalar=b1_sb[:, c2 : c2 + 1],
                    in1=sg_sb[:, c2, :],
                    op0=ALU.add,
                    op1=ALU.mult,
                )

            low_ps = ps_low.tile([ER, TT], F32, tag="low")
            for c in range(HC):
                nc.tensor.matmul(
                    out=low_ps,
                    lhsT=acat_sb[:, c, :, :],
                    rhs=xt_r[:, c, :],
                    start=(c == 0),
                    stop=(c == HC - 1),
                )
            st["low_ps"] = low_ps

            # token-major logits for the 4 q-halves, packed 2-per-rt-half so
            # each [128, E] matmul output stays inside one PSUM bank
            lg_full = ps_rt.tile([128, RC, TT], F32, tag="rt")
            lg4 = lg_full[:, :, 0 : 2 * E].rearrange("p a (b e) -> p a b e", e=E)
            for q in range(JT):
                for c2 in range(RC):
                    nc.tensor.matmul(
                        out=lg4[:, q // 2, q % 2, :],
                        lhsT=ht_sb[:, c2, ts(q, 128)],
                        rhs=w2_sb[:, c2, :],
                        start=(c2 == 0),
                        stop=False,
                    )
                nc.tensor.matmul(
                    out=lg4[:, q // 2, q % 2, :],
                    lhsT=ones_sb,
                    rhs=b2_sb,
                    start=False,
                    stop=True,
                )

            # top-2 fused over all 4 q-halves via [128, 2, 2, E] views
            m1 = small_p.tile([128, JT], F32, tag="m1")
            m1_4 = m1.rearrange("p (a b) -> p a b", b=2)
            nc.vector.tensor_reduce(out=m1_4, in_=lg4, axis=AX.X, op=ALU.max)
            top1 = small_p.tile([128, JT, E], F32, tag="top1")
            top1_4 = top1.rearrange("p (a b) e -> p a b e", b=2)
            nc.vector.tensor_tensor(
                out=top1_4,
                in0=lg4,
                in1=m1_4.unsqueeze(-1).broadcast_to([128, 2, 2, E]),
                op=ALU.is_equal,
            )
            masked = small_p.tile([128, JT, E], F32, tag="masked")
            masked_4 = masked.rearrange("p (a b) e -> p a b e", b=2)
            nc.vector.scalar_tensor_tensor(
                out=masked_4, in0=top1_4, scalar=-1e30, in1=lg4,
                op0=ALU.mult, op1=ALU.add,
            )
            m2 = small_p.tile([128, JT], F32, tag="m2")
            nc.vector.tensor_reduce(out=m2, in_=masked, axis=AX.X, op=ALU.max)
            dlg = small_p.tile([128, JT], F32, tag="dlg")
            nc.vector.tensor_tensor(out=dlg, in0=m2, in1=m1, op=ALU.subtract)
            st["top1"], st["masked"], st["m2"], st["dlg"] = top1, masked, m2, dlg

        def emit_weights(st):
            top1, masked, m2, dlg = st["top1"], st["masked"], st["m2"], st["dlg"]
            u2 = small_p.tile([128, JT], F32, tag="u2")
            nc.scalar.activation(out=u2, in_=dlg, func=AF.Sigmoid)
            u1 = small_p.tile([128, JT], F32, tag="u1")
            nc.vector.tensor_scalar(
                out=u1, in0=u2, scalar1=-1.0, scalar2=1.0,
                op0=ALU.mult, op1=ALU.add,
            )
            top2 = small_p.tile([128, JT, E], F32, tag="top2")
            nc.vector.tensor_tensor(
                out=top2,
                in0=masked,
                in1=m2.unsqueeze(-1).broadcast_to([128, JT, E]),
                op=ALU.is_equal,
            )
            w_full = small_p.tile([128, JT, ER], F32R)
            wt2 = small_p.tile([128, JT, ER], F32, tag="wt2")
            w4 = w_full.rearrange("p j (e r) -> p j e r", r=R)
            wt24 = wt2.rearrange("p j (e r) -> p j e r", r=R)
            t14 = top1.unsqueeze(-1).broadcast_to([128, JT, E, R])
            t24 = top2.unsqueeze(-1).broadcast_to([128, JT, E, R])
            u14 = u1.unsqueeze(-1).unsqueeze(-1).broadcast_to([128, JT, E, R])
            u24 = u2.unsqueeze(-1).unsqueeze(-1).broadcast_to([128, JT, E, R])
            nc.vector.tensor_tensor(out=wt24, in0=t24, in1=u24, op=ALU.mult)
            nc.vector.tensor_tensor(out=w4, in0=t14, in1=u14, op=ALU.mult)
            nc.vector.tensor_tensor(out=w_full, in0=w_full, in1=wt2, op=ALU.add)
            st["w_full"] = w_full

        def emit_m(st):
            w_full = st["w_full"]
            wrt_ps = ps_wrt.tile([ER, JT, 128], F32R, tag="wrt")
            for q in range(JT):
                nc.tensor.transpose(
                    out=wrt_ps[:, q, :],
                    in_=w_full[:, q, :],
                    identity=ident_r,
                )
            wrt_sb = small_p.tile([ER, JT, 128], F32R, tag="wrt_sb")
            nc.any.tensor_copy(out=wrt_sb, in_=wrt_ps)
            lw_sb = lw_p.tile([ER, TT], F32R)
            nc.vector.scalar_tensor_tensor(
                out=lw_sb,
                in0=st["low_ps"],
                scalar=float(SCALING),
                in1=wrt_sb.rearrange("p j t -> p (j t)"),
                op0=ALU.mult,
                op1=ALU.mult,
            )
            st["lw_sb"] = lw_sb

        def emit_delta(st):
            lw_r = st["lw_sb"]
            bo = bout_p.tile([128, JT, H], F32)
            st["bo"] = bo
            for q in range(JT):
                dla = ps_dl.tile([128, 320], F32, tag="dla")
                dlb = ps_dl.tile([128, 320], F32, tag="dlb")
                nc.tensor.matmul(
                    out=dla, lhsT=lw_r[:, ts(q, 128)], rhs=bcat_sb[:, 0:320],
                    start=True, stop=True,
                )
                nc.tensor.matmul(
                    out=dlb, lhsT=lw_r[:, ts(q, 128)], rhs=bcat_sb[:, 320:H],
                    start=True, stop=True,
                )
                nc.any.tensor_copy(out=bo[:, q, 0:320], in_=dla)
                nc.any.tensor_copy(out=bo[:, q, 320:H], in_=dlb)

        def emit_accum(st):
            nc.gpsimd.dma_start(
                out=st["bo"],
                in_=base_d[st["tok"] : st["tok"] + TT, :].rearrange(
                    "(q p) h -> p q h", p=128
                ),
                accum_op=ALU.add,
            )

        def emit_store(st):
            nc.scalar.dma_start(
                out=out_d[st["tok"] : st["tok"] + TT, :].rearrange(
                    "(q p) h -> p q h", p=128
                ),
                in_=st["bo"],
            )

        hist = []
        for p in range(passes):
            for i in range(ntiles):
                st = emit_front(i)
                emit_router(st)
                emit_weights(st)
                emit_m(st)
                hist.append(st)
                if len(hist) >= 2:
                    emit_delta(hist[-2])
                if len(hist) >= 3:
                    emit_accum(hist[-3])
                if len(hist) >= 4:
                    emit_store(hist[-4])
        emit_delta(hist[-1])
        if len(hist) >= 2:
            emit_accum(hist[-2])
        emit_accum(hist[-1])
        if len(hist) >= 3:
            emit_store(hist[-3])
        if len(hist) >= 2:
            emit_store(hist[-2])
        emit_store(hist[-1])

    return nc


_CACHE = {}


def _get_nc(t_core=T_CORE, niter=1, timing_mode=False, passes=1):
    key = (t_core, niter, timing_mode, passes)
    if key not in _CACHE:
        nc = build_kernel(t_core, niter, timing_mode, passes)
        nc.finalize()
        _CACHE[key] = nc
    return _CACHE[key]


def kernel(x, base_output, W1, b1, W2, b2, A, Bm):
    x = np.ascontiguousarray(np.asarray(x), dtype=np.float32)
    base_output = np.ascontiguousarray(np.asarray(base_output), dtype=np.float32)
    W1 = np.ascontiguousarray(np.asarray(W1), dtype=np.float32)
    b1 = np.ascontiguousarray(np.asarray(b1), dtype=np.float32)
    W2 = np.ascontiguousarray(np.asarray(W2), dtype=np.float32)
    b2 = np.ascontiguousarray(np.asarray(b2), dtype=np.float32)
    A = np.ascontiguousarray(np.asarray(A), dtype=np.float32)
    Bm = np.ascontiguousarray(np.asarray(Bm), dtype=np.float32)

    B, S, _ = x.shape
    assert B * S == N_CORES * T_CORE
    xs = x.reshape(N_CORES, T_CORE, H)
    bs = base_output.reshape(N_CORES, T_CORE, H)

    nc = _get_nc()
    in_maps = [
        {
            "x": np.ascontiguousarray(xs[i]),
            "base": np.ascontiguousarray(bs[i]),
            "W1": W1, "b1": b1, "W2": W2, "b2": b2, "A": A, "Bm": Bm,
        }
        for i in range(N_CORES)
    ]
    res = run_bass_kernel_spmd(nc, in_maps, list(range(N_CORES))).results
    out = np.stack([res[i]["out"] for i in range(N_CORES)], axis=0)
    return out.reshape(B, S, H).astype(np.float32)



# revision 4
# speedup vs baseline: 2.2385x; 2.2385x over previous
"""MoLoRA Trainium2 Bass kernel — r25.

Design (fp16 compute, rel-err ~5e-3 vs gate 2e-2):
- Host prep: shard per core, downcast to fp16, pre-transpose x to x^T [H,T]
  so the device does plain contiguous DMA loads (no transposes on device).
- Router SiLU fused into one ACT Silu activation (PSUM source).
- Top-k small ops split: PSUM-readers on DVE, SBUF-only ops on gpsimd.
- Delta + base: for q<2 PE preloads base into PSUM (identity matmul) and the
  delta matmul accumulates, evacuated by ACT Copy; for q>=2 plain delta
  matmul evacuated by a DVE add with the base tile. Balances PE/ACT/DVE.
- Software-pipelined emission: per step s PE runs router(s), logits(s-1),
  wrt(s-2), low(s-1), delta(s-3); store(s-4) on the ACT HWDGE ring. Each
  engine's program order only ever waits on results from earlier steps.
- Output fp16 from device, upcast to f32 on host (fp16_out=True).
"""

import numpy as np
from contextlib import ExitStack

import concourse.bass as bass
import concourse.tile as tile
from concourse import bacc
from concourse import mybir
from concourse.bass import ts
from concourse.masks import make_identity
from concourse.bass_utils import run_bass_kernel_spmd

F32 = mybir.dt.float32
FP16 = mybir.dt.float16
AF = mybir.ActivationFunctionType
ALU = mybir.AluOpType
AX = mybir.AxisListType

H = 640
E = 5
R = 8
ER = E * R
RH = 256
HC = H // 128
RC = RH // 128
SCALING = 16.0 / R
N_CORES = 8
T_CORE = 4096
TT = 512          # compute tile
JT = TT // 128    # 4
XB = 1024         # x^T load-chunk tokens
FP16_OUT = True

# host-packed fp16 const blob layout (per partition)
OFF_W1 = 0                      # [HC, RH]   w1_sb[p, c, m] = W1[c*128+p, m]
OFF_A = OFF_W1 + HC * RH        # [HC, E, R] acat[p, c, e, r] = A[e, c*128+p, r]
OFF_W2 = OFF_A + HC * E * R     # [RC, E]    w2[p, c2, e] = W2[c2*128+p, e]
OFF_B = OFF_W2 + RC * E         # rows 0..ER: bcat[er, h] = Bm[e, r, h]
OFF_B2 = OFF_B + H              # row 0: b2 tiled JT times
NCONST = OFF_B2 + JT * E


def build_kernel(t_core=T_CORE, niter=1, timing_mode=False, mode="full",
                 fp16_out=FP16_OUT):
    ntiles = t_core // TT
    tiles_per_xc = XB // TT
    nxc = t_core // XB
    OUT_DT = FP16 if fp16_out else F32
    nc = bacc.Bacc()

    if timing_mode:
        xT_d = nc.dram_tensor("xT_int", [H, t_core], FP16)[:, :]
        bb_d = nc.dram_tensor("bb_int", [t_core, H], FP16)[:, :]
        out_d = nc.dram_tensor("out_int", [t_core, H], OUT_DT)[:, :]
        dummy_d = nc.declare_dram_parameter("dummy_out", [1, 4], F32, isOutput=True)
        cb_d = nc.dram_tensor("cb_int", [128, NCONST], FP16)[:, :]
        b1_d = nc.dram_tensor("b1_int", [128, RC], F32)[:, :]
    else:
        xT_d = nc.declare_dram_parameter("xT", [H, t_core], FP16, isOutput=False)[:, :]
        bb_d = nc.declare_dram_parameter("bb", [t_core, H], FP16, isOutput=False)[:, :]
        out_d = nc.declare_dram_parameter("out", [t_core, H], OUT_DT, isOutput=True)[:, :]
        dummy_d = None
        cb_d = nc.declare_dram_parameter("cb", [128, NCONST], FP16, isOutput=False)[:, :]
        b1_d = nc.declare_dram_parameter("b1", [128, RC], F32, isOutput=False)[:, :]

    with ExitStack() as ctx:
        tc = ctx.enter_context(tile.TileContext(nc))
        const = ctx.enter_context(tc.tile_pool(name="const", bufs=1))
        bs_p = ctx.enter_context(tc.tile_pool(name="bs", bufs=4))
        bout_p = ctx.enter_context(tc.tile_pool(name="bout", bufs=3))
        xt_p = ctx.enter_context(tc.tile_pool(name="xt", bufs=3))
        ht_p = ctx.enter_context(tc.tile_pool(name="ht", bufs=2))
        small_p = ctx.enter_context(tc.tile_pool(name="small", bufs=4))
        lw_p = ctx.enter_context(tc.tile_pool(name="lw", bufs=3))
        ps_rt = ctx.enter_context(tc.tile_pool(name="ps_rt", bufs=1, space="PSUM"))
        ps_lg = ctx.enter_context(tc.tile_pool(name="ps_lg", bufs=1, space="PSUM"))
        ps_low = ctx.enter_context(tc.tile_pool(name="ps_low", bufs=2, space="PSUM"))
        ps_wrt = ctx.enter_context(tc.tile_pool(name="ps_wrt", bufs=1, space="PSUM"))
        ps_dl = ctx.enter_context(tc.tile_pool(name="ps_dl", bufs=1, space="PSUM"))

        ident = const.tile([128, 128], F32)
        make_identity(nc, ident)
        ident_h = const.tile([128, 128], FP16)
        nc.vector.tensor_copy(out=ident_h, in_=ident)

        # all fp16 constants arrive host-packed in one blob -> one fast DMA
        cb = const.tile([128, NCONST], FP16)
        nc.sync.dma_start(out=cb, in_=cb_d)
        b1_sb = const.tile([128, RC], F32)
        nc.sync.dma_start(out=b1_sb, in_=b1_d)
        w1_sb = cb[:, OFF_W1 : OFF_W1 + HC * RH].rearrange("p (c m) -> p c m", c=HC)
        acat_sb = cb[:, OFF_A : OFF_A + HC * E * R].rearrange(
            "p (c e r) -> p c e r", c=HC, e=E)
        w2_sb = cb[:, OFF_W2 : OFF_W2 + RC * E].rearrange("p (c e) -> p c e", c=RC)
        bcat_sb = cb[0:ER, OFF_B : OFF_B + H]
        b2rep_sb = cb[0:1, OFF_B2 : OFF_B2 + JT * E].rearrange(
            "p (q e) -> p q e", q=JT)
        ones_sb = const.tile([1, 128], FP16)
        nc.vector.memset(ones_sb, 1.0)

        if mode != "full":
            zeros16 = const.tile([128, JT, H], FP16)
            nc.vector.memset(zeros16, 0.0)
            zeros_out = const.tile([128, JT, H], OUT_DT)
            nc.vector.memset(zeros_out, 0.0)

        xt_res = None
        if mode == "comp":
            xt_res = const.tile([128, HC, TT], FP16)
            nc.sync.dma_start(
                out=xt_res,
                in_=xT_d[:, 0:TT].rearrange("(c p) t -> p c t", p=128),
            )

        if dummy_d is not None:
            dnm = const.tile([1, 4], F32)
            nc.vector.memset(dnm, 1.0)
            nc.sync.dma_start(out=dummy_d[:, :], in_=dnm)

        loop_ctx = tc.For_i(0, niter, 1) if niter > 1 else None
        if loop_ctx is not None:
            ctx.enter_context(loop_ctx)

        dma_only = mode == "dma"
        comp_only = mode == "comp"
        st_by_tile = {}

        def emit_xchunk(g):
            xt_sb = xt_p.tile([128, HC, XB], FP16)
            nc.sync.dma_start(
                out=xt_sb,
                in_=xT_d[:, g * XB : (g + 1) * XB].rearrange(
                    "(c p) t -> p c t", p=128
                ),
            )
            return xt_sb

        def emit_bs(i):
            bs = bs_p.tile([128, JT, H], FP16)
            nc.sync.dma_start(
                out=bs,
                in_=bb_d[i * TT : (i + 1) * TT, :].rearrange(
                    "(q p) h -> p q h", p=128
                ),
            )
            st_by_tile[i]["bs"] = bs

        def emit_router_half(i, c2):
            st = st_by_tile[i]
            xt_r = st["xt_sb"]
            if c2 == 0:
                h_ps = ps_rt.tile([128, RC, TT], F32, tag="rt")
                st["h_ps"] = h_ps
            h_ps = st["h_ps"]
            for c in range(HC):
                nc.tensor.matmul(
                    out=h_ps[:, c2, :],
                    lhsT=w1_sb[:, c, ts(c2, 128)],
                    rhs=xt_r[:, c, :],
                    start=(c == 0),
                    stop=(c == HC - 1),
                )

        def emit_ht(i):
            st = st_by_tile[i]
            h_ps = st["h_ps"]
            ht_sb = ht_p.tile([128, RC, TT], FP16)
            for c2 in range(RC):
                nc.scalar.activation(
                    out=ht_sb[:, c2, :],
                    in_=h_ps[:, c2, :],
                    func=AF.Silu,
                    bias=b1_sb[:, c2 : c2 + 1],
                )
            st["ht_sb"] = ht_sb

        def emit_low(i):
            st = st_by_tile[i]
            xt_r = st["xt_sb"]
            low_ps = ps_low.tile([ER, TT], F32, tag="low")
            for c in range(HC):
                nc.tensor.matmul(
                    out=low_ps,
                    lhsT=acat_sb[:, c, :, :],
                    rhs=xt_r[:, c, :],
                    start=(c == 0),
                    stop=(c == HC - 1),
                )
            st["low_ps"] = low_ps

        def emit_lg(i):
            st = st_by_tile[i]
            ht_sb = st["ht_sb"]
            lg = ps_lg.tile([128, JT, E], F32, tag="lg")
            st["lg"] = lg
            for q in range(JT):
                for c2 in range(RC):
                    nc.tensor.matmul(
                        out=lg[:, q, :],
                        lhsT=ht_sb[:, c2, ts(q, 128)],
                        rhs=w2_sb[:, c2, :],
                        start=(c2 == 0),
                        stop=False,
                    )
                nc.tensor.matmul(
                    out=lg[:, q, :],
                    lhsT=ones_sb,
                    rhs=b2rep_sb[:, q, :],
                    start=False,
                    stop=True,
                )

        def emit_topk_a(i):
            st = st_by_tile[i]
            lg = st["lg"]
            # PSUM readers on DVE
            m1 = small_p.tile([128, JT], F32, tag="m1")
            nc.vector.tensor_reduce(out=m1, in_=lg, axis=AX.X, op=ALU.max)
            top1 = small_p.tile([128, JT, E], F32, tag="top1")
            nc.vector.tensor_tensor(
                out=top1,
                in0=lg,
                in1=m1.unsqueeze(-1).broadcast_to([128, JT, E]),
                op=ALU.is_equal,
            )
            masked = small_p.tile([128, JT, E], F32, tag="masked")
            nc.vector.scalar_tensor_tensor(
                out=masked, in0=top1, scalar=-1e30, in1=lg,
                op0=ALU.mult, op1=ALU.add,
            )
            # SBUF-only ops on gpsimd (Pool)
            m2 = small_p.tile([128, JT], F32, tag="m2")
            nc.vector.tensor_reduce(out=m2, in_=masked, axis=AX.X, op=ALU.max)
            dlg = small_p.tile([128, JT], F32, tag="dlg")
            nc.gpsimd.tensor_tensor(out=dlg, in0=m2, in1=m1, op=ALU.subtract)
            # sigmoid(d) = 0.5 + 0.5*tanh(d/2); tanh shares the silu ACT
            # table set, so the steady loop never reloads the function table
            th = small_p.tile([128, JT], F32, tag="th")
            nc.scalar.activation(out=th, in_=dlg, func=AF.Tanh, scale=0.5)
            st["m1"], st["m2"], st["masked"], st["top1"], st["th"] = \
                m1, m2, masked, top1, th

        def emit_topk_b(i):
            st = st_by_tile[i]
            m1, m2, masked, top1, th = (st["m1"], st["m2"], st["masked"],
                                        st["top1"], st["th"])
            u2 = small_p.tile([128, JT], F32, tag="u2")
            nc.gpsimd.tensor_scalar(
                out=u2, in0=th, scalar1=0.5, scalar2=0.5,
                op0=ALU.mult, op1=ALU.add,
            )
            u1 = small_p.tile([128, JT], F32, tag="u1")
            nc.gpsimd.tensor_scalar(
                out=u1, in0=th, scalar1=-0.5, scalar2=0.5,
                op0=ALU.mult, op1=ALU.add,
            )
            top2 = small_p.tile([128, JT, E], F32, tag="top2")
            nc.vector.tensor_tensor(
                out=top2,
                in0=masked,
                in1=m2.unsqueeze(-1).broadcast_to([128, JT, E]),
                op=ALU.is_equal,
            )
            w_full = small_p.tile([128, JT, ER], FP16)
            wt2 = small_p.tile([128, JT, ER], F32, tag="wt2")
            w4 = w_full.rearrange("p j (e r) -> p j e r", r=R)
            wt24 = wt2.rearrange("p j (e r) -> p j e r", r=R)
            t14 = top1.unsqueeze(-1).broadcast_to([128, JT, E, R])
            t24 = top2.unsqueeze(-1).broadcast_to([128, JT, E, R])
            u14 = u1.unsqueeze(-1).unsqueeze(-1).broadcast_to([128, JT, E, R])
            u24 = u2.unsqueeze(-1).unsqueeze(-1).broadcast_to([128, JT, E, R])
            nc.vector.tensor_tensor(out=wt24, in0=t24, in1=u24, op=ALU.mult)
            nc.vector.tensor_tensor(out=w4, in0=t14, in1=u14, op=ALU.mult)
            nc.gpsimd.tensor_tensor(out=w_full, in0=w_full, in1=wt2, op=ALU.add)
            st["w_full"] = w_full

        def emit_wrt(i):
            st = st_by_tile[i]
            w_full = st["w_full"]
            wrt_ps = ps_wrt.tile([ER, JT, 128], FP16, tag="wrt")
            for q in range(JT):
                nc.tensor.transpose(
                    out=wrt_ps[:, q, :],
                    in_=w_full[:, q, :],
                    identity=ident_h,
                )
            wrt_sb = small_p.tile([ER, JT, 128], FP16, tag="wrt_sb")
            nc.vector.tensor_copy(out=wrt_sb, in_=wrt_ps)
            lw_sb = lw_p.tile([ER, TT], FP16)
            nc.vector.scalar_tensor_tensor(
                out=lw_sb,
                in0=st["low_ps"],
                scalar=float(SCALING),
                in1=wrt_sb.rearrange("p j t -> p (j t)"),
                op0=ALU.mult,
                op1=ALU.mult,
            )
            st["lw_sb"] = lw_sb

        def emit_delta_q(i, q):
            """q<2: PE preloads base into PSUM, delta accumulates, ACT Copy
            evacuates. q>=2: plain delta matmul, DVE add with base tile.
            Emitted per q, interleaved with router halves in PE program
            order so the bufs=1 dla/dlb WAR hides behind router streams."""
            st = st_by_tile[i]
            lw_r = st["lw_sb"]
            bs = st["bs"]
            if q == 0:
                bo = bout_p.tile([128, JT, H], OUT_DT)
                st["bo"] = bo
            bo = st["bo"]
            if True:
                dla = ps_dl.tile([128, 320], F32, tag="dla")
                dlb = ps_dl.tile([128, 320], F32, tag="dlb")
                pre = q < 2
                if pre:
                    nc.tensor.matmul(
                        out=dla, lhsT=ident_h, rhs=bs[:, q, 0:320],
                        start=True, stop=False,
                    )
                    nc.tensor.matmul(
                        out=dlb, lhsT=ident_h, rhs=bs[:, q, 320:H],
                        start=True, stop=False,
                    )
                nc.tensor.matmul(
                    out=dla, lhsT=lw_r[:, ts(q, 128)], rhs=bcat_sb[:, 0:320],
                    start=not pre, stop=True,
                )
                nc.tensor.matmul(
                    out=dlb, lhsT=lw_r[:, ts(q, 128)], rhs=bcat_sb[:, 320:H],
                    start=not pre, stop=True,
                )
                if pre:
                    nc.scalar.activation(
                        out=bo[:, q, 0:320], in_=dla, func=AF.Copy)
                    nc.scalar.activation(
                        out=bo[:, q, 320:H], in_=dlb, func=AF.Copy)
                else:
                    nc.vector.tensor_tensor(
                        out=bo[:, q, 0:320], in0=dla, in1=bs[:, q, 0:320],
                        op=ALU.add)
                    nc.vector.tensor_tensor(
                        out=bo[:, q, 320:H], in0=dlb, in1=bs[:, q, 320:H],
                        op=ALU.add)

        def emit_store(i):
            st = st_by_tile[i]
            nc.scalar.dma_start(
                out=out_d[i * TT : (i + 1) * TT, :].rearrange(
                    "(q p) h -> p q h", p=128
                ),
                in_=st["bo"],
            )

        xcs = {}
        if not comp_only:
            for g in range(min(2, nxc)):
                xcs[g] = emit_xchunk(g)

        nsteps = ntiles + 4
        for s in range(nsteps):
            g_need = (s + 2) // tiles_per_xc
            if not comp_only:
                for g in range(len(xcs), min(g_need + 1, nxc)):
                    xcs[g] = emit_xchunk(g)
            if s < ntiles:
                g, j = divmod(s, tiles_per_xc)
                st_by_tile[s] = {
                    "xt_sb": (xt_res if comp_only
                              else xcs[g][:, :, j * TT : (j + 1) * TT]),
                }
                if comp_only:
                    st_by_tile[s]["bs"] = zeros16
                else:
                    emit_bs(s)

            if dma_only:
                if s < ntiles:
                    st_by_tile[s]["bo"] = zeros_out
                    emit_store(s)
                continue

            if s < ntiles:
                emit_router_half(s, 0)
                emit_router_half(s, 1)
            if 0 <= s - 1 < ntiles:
                emit_lg(s - 1)
                emit_topk_a(s - 1)
            if 0 <= s - 2 < ntiles:
                emit_wrt(s - 2)
            if 0 <= s - 1 < ntiles:
                emit_low(s - 1)
                emit_topk_b(s - 1)
            if s < ntiles:
                emit_ht(s)
            if 0 <= s - 3 < ntiles:
                for q in range(JT):
                    emit_delta_q(s - 3, q)
            if not comp_only and 0 <= s - 4 < ntiles:
                emit_store(s - 4)

    return nc


_CACHE = {}


def _get_nc(t_core=T_CORE, niter=1, timing_mode=False, mode="full",
            fp16_out=FP16_OUT):
    key = (t_core, niter, timing_mode, mode, fp16_out)
    if key not in _CACHE:
        nc = build_kernel(t_core, niter, timing_mode, mode, fp16_out)
        nc.finalize()
        _CACHE[key] = nc
    return _CACHE[key]


def kernel(x, base_output, W1, b1, W2, b2, A, Bm):
    x = np.asarray(x)
    base_output = np.asarray(base_output)
    B, S, _ = x.shape
    assert B * S == N_CORES * T_CORE

    # host-side prep: shard per core, downcast to fp16, pre-transpose x,
    # pack all small constants into one [128, NCONST] blob (one DMA on device)
    xs = x.reshape(N_CORES, T_CORE, H)
    xT = np.ascontiguousarray(xs.transpose(0, 2, 1)).astype(np.float16)
    bb = base_output.reshape(N_CORES, T_CORE, H).astype(np.float16)
    W1h = np.asarray(W1, dtype=np.float16)
    W2h = np.asarray(W2, dtype=np.float16)
    Ah = np.asarray(A, dtype=np.float16)
    Bmh = np.asarray(Bm, dtype=np.float16)
    b2h = np.asarray(b2, dtype=np.float16)

    cb = np.zeros((128, NCONST), np.float16)
    cb[:, OFF_W1:OFF_W1 + HC * RH] = (
        W1h.reshape(HC, 128, RH).transpose(1, 0, 2).reshape(128, HC * RH))
    cb[:, OFF_A:OFF_A + HC * E * R] = (
        Ah.reshape(E, HC, 128, R).transpose(2, 1, 0, 3).reshape(128, HC * E * R))
    cb[:, OFF_W2:OFF_W2 + RC * E] = (
        W2h.reshape(RC, 128, E).transpose(1, 0, 2).reshape(128, RC * E))
    cb[0:ER, OFF_B:OFF_B + H] = Bmh.reshape(ER, H)
    cb[0, OFF_B2:OFF_B2 + JT * E] = np.tile(b2h, JT)
    b1f = np.ascontiguousarray(
        np.asarray(b1, dtype=np.float32).reshape(RC, 128).T)

    nc = _get_nc()
    in_maps = [
        {
            "xT": xT[i],
            "bb": np.ascontiguousarray(bb[i]),
            "cb": cb, "b1": b1f,
        }
        for i in range(N_CORES)
    ]
    res = run_bass_kernel_spmd(nc, in_maps, list(range(N_CORES))).results
    out = np.stack([res[i]["out"] for i in range(N_CORES)], axis=0)
    return out.reshape(B, S, H).astype(np.float32)


# revision 5
# speedup vs baseline: 2.2947x; 1.0251x over previous
"""MoLoRA Trainium2 Bass kernel — r27.

Design (fp16 compute, rel-err ~5e-3 vs gate 2e-2):
- Host prep: shard per core, downcast to fp16, pre-transpose x to x^T [H,T]
  so the device does plain contiguous DMA loads (no transposes on device).
- Router SiLU fused into one ACT Silu activation (PSUM source).
- Top-k small ops split: PSUM-readers on DVE, SBUF-only ops on gpsimd.
- Router bias b2 folded into a host-replicated broadcast constant added on
  DVE (no bias matmuls; top-k reads SBUF instead of PSUM).
- Delta + base: for q<2 PE preloads base into PSUM (identity matmul) and the
  delta matmul accumulates, evacuated by ACT Copy; for q>=2 plain delta
  matmul evacuated by a DVE add with the base tile. Balances PE/ACT/DVE.
- Software-pipelined emission: per step s PE runs router(s), wrt(s-2),
  low(s-1)+logits(s-1), delta(s-3); store(s-4) on the ACT HWDGE ring.
- Output fp16 from device, upcast to f32 on host (fp16_out=True).
"""

import numpy as np
from contextlib import ExitStack

import concourse.bass as bass
import concourse.tile as tile
from concourse import bacc
from concourse import mybir
from concourse.bass import ts
from concourse.masks import make_identity
from concourse.bass_utils import run_bass_kernel_spmd

F32 = mybir.dt.float32
FP16 = mybir.dt.float16
AF = mybir.ActivationFunctionType
ALU = mybir.AluOpType
AX = mybir.AxisListType

H = 640
E = 5
R = 8
ER = E * R
RH = 256
HC = H // 128
RC = RH // 128
SCALING = 16.0 / R
N_CORES = 8
T_CORE = 4096
TT = 512          # compute tile
JT = TT // 128    # 4
XB = 1024         # x^T load-chunk tokens
FP16_OUT = True

# host-packed fp16 const blob layout (per partition)
OFF_W1 = 0                      # [HC, RH]     w1_sb[p, c, m] = W1[c*128+p, m]
OFF_A = OFF_W1 + HC * RH        # [HC, E, R]   acat[p, c, e, r] = A[e, c*128+p, r]
OFF_W2 = OFF_A + HC * E * R     # [RC, E]      w2[p, c2, e] = W2[c2*128+p, e]
OFF_B = OFF_W2 + RC * E         # rows 0..ER: bcat[er, h] = Bm[e, r, h]
OFF_B2 = OFF_B + H              # all rows: b2 tiled JT times (broadcast add)
NCONST = OFF_B2 + JT * E


def build_kernel(t_core=T_CORE, niter=1, timing_mode=False, mode="full",
                 fp16_out=FP16_OUT):
    ntiles = t_core // TT
    tiles_per_xc = XB // TT
    nxc = t_core // XB
    OUT_DT = FP16 if fp16_out else F32
    nc = bacc.Bacc()

    if timing_mode:
        xT_d = nc.dram_tensor("xT_int", [H, t_core], FP16)[:, :]
        bb_d = nc.dram_tensor("bb_int", [t_core, H], FP16)[:, :]
        out_d = nc.dram_tensor("out_int", [t_core, H], OUT_DT)[:, :]
        dummy_d = nc.declare_dram_parameter("dummy_out", [1, 4], F32, isOutput=True)
        cb_d = nc.dram_tensor("cb_int", [128, NCONST], FP16)[:, :]
        b1_d = nc.dram_tensor("b1_int", [128, RC], F32)[:, :]
    else:
        xT_d = nc.declare_dram_parameter("xT", [H, t_core], FP16, isOutput=False)[:, :]
        bb_d = nc.declare_dram_parameter("bb", [t_core, H], FP16, isOutput=False)[:, :]
        out_d = nc.declare_dram_parameter("out", [t_core, H], OUT_DT, isOutput=True)[:, :]
        dummy_d = None
        cb_d = nc.declare_dram_parameter("cb", [128, NCONST], FP16, isOutput=False)[:, :]
        b1_d = nc.declare_dram_parameter("b1", [128, RC], F32, isOutput=False)[:, :]

    with ExitStack() as ctx:
        tc = ctx.enter_context(tile.TileContext(nc))
        const = ctx.enter_context(tc.tile_pool(name="const", bufs=1))
        bs_p = ctx.enter_context(tc.tile_pool(name="bs", bufs=4))
        bout_p = ctx.enter_context(tc.tile_pool(name="bout", bufs=3))
        xt_p = ctx.enter_context(tc.tile_pool(name="xt", bufs=3))
        ht_p = ctx.enter_context(tc.tile_pool(name="ht", bufs=2))
        small_p = ctx.enter_context(tc.tile_pool(name="small", bufs=4))
        lw_p = ctx.enter_context(tc.tile_pool(name="lw", bufs=3))
        ps_rt = ctx.enter_context(tc.tile_pool(name="ps_rt", bufs=1, space="PSUM"))
        ps_lg = ctx.enter_context(tc.tile_pool(name="ps_lg", bufs=1, space="PSUM"))
        ps_low = ctx.enter_context(tc.tile_pool(name="ps_low", bufs=2, space="PSUM"))
        ps_wrt = ctx.enter_context(tc.tile_pool(name="ps_wrt", bufs=1, space="PSUM"))
        ps_dl = ctx.enter_context(tc.tile_pool(name="ps_dl", bufs=1, space="PSUM"))

        ident = const.tile([128, 128], F32)
        make_identity(nc, ident)
        ident_h = const.tile([128, 128], FP16)
        nc.vector.tensor_copy(out=ident_h, in_=ident)

        # all fp16 constants arrive host-packed in one blob -> one fast DMA
        cb = const.tile([128, NCONST], FP16)
        nc.sync.dma_start(out=cb, in_=cb_d)
        b1_sb = const.tile([128, RC], F32)
        nc.sync.dma_start(out=b1_sb, in_=b1_d)
        w1_sb = cb[:, OFF_W1 : OFF_W1 + HC * RH].rearrange("p (c m) -> p c m", c=HC)
        acat_sb = cb[:, OFF_A : OFF_A + HC * E * R].rearrange(
            "p (c d) -> p c d", c=HC)
        w2_sb = cb[:, OFF_W2 : OFF_W2 + RC * E].rearrange("p (c e) -> p c e", c=RC)
        bcat_sb = cb[0:ER, OFF_B : OFF_B + H]
        b2bc_sb = cb[:, OFF_B2 : OFF_B2 + JT * E].rearrange(
            "p (q e) -> p q e", q=JT)

        if mode != "full":
            zeros16 = const.tile([128, JT, H], FP16)
            nc.vector.memset(zeros16, 0.0)
            zeros_out = const.tile([128, JT, H], OUT_DT)
            nc.vector.memset(zeros_out, 0.0)

        xt_res = None
        if mode == "comp":
            xt_res = const.tile([128, HC, TT], FP16)
            nc.sync.dma_start(
                out=xt_res,
                in_=xT_d[:, 0:TT].rearrange("(c p) t -> p c t", p=128),
            )

        if dummy_d is not None:
            dnm = const.tile([1, 4], F32)
            nc.vector.memset(dnm, 1.0)
            nc.sync.dma_start(out=dummy_d[:, :], in_=dnm)

        loop_ctx = tc.For_i(0, niter, 1) if niter > 1 else None
        if loop_ctx is not None:
            ctx.enter_context(loop_ctx)

        dma_only = mode == "dma"
        comp_only = mode == "comp"
        st_by_tile = {}

        def emit_xchunk(g):
            xt_sb = xt_p.tile([128, HC, XB], FP16)
            nc.sync.dma_start(
                out=xt_sb,
                in_=xT_d[:, g * XB : (g + 1) * XB].rearrange(
                    "(c p) t -> p c t", p=128
                ),
            )
            return xt_sb

        def emit_bs(i):
            bs = bs_p.tile([128, JT, H], FP16)
            nc.sync.dma_start(
                out=bs,
                in_=bb_d[i * TT : (i + 1) * TT, :].rearrange(
                    "(q p) h -> p q h", p=128
                ),
            )
            st_by_tile[i]["bs"] = bs

        def emit_router_half(i, c2):
            st = st_by_tile[i]
            xt_r = st["xt_sb"]
            if c2 == 0:
                h_ps = ps_rt.tile([128, RC, TT], F32, tag="rt")
                st["h_ps"] = h_ps
            h_ps = st["h_ps"]
            for c in range(HC):
                nc.tensor.matmul(
                    out=h_ps[:, c2, :],
                    lhsT=w1_sb[:, c, ts(c2, 128)],
                    rhs=xt_r[:, c, :],
                    start=(c == 0),
                    stop=(c == HC - 1),
                )

        def emit_ht(i):
            st = st_by_tile[i]
            h_ps = st["h_ps"]
            ht_sb = ht_p.tile([128, RC, TT], FP16)
            for c2 in range(RC):
                nc.scalar.activation(
                    out=ht_sb[:, c2, :],
                    in_=h_ps[:, c2, :],
                    func=AF.Silu,
                    bias=b1_sb[:, c2 : c2 + 1],
                )
            st["ht_sb"] = ht_sb

        def emit_low(i):
            st = st_by_tile[i]
            xt_r = st["xt_sb"]
            low_ps = ps_low.tile([ER, TT], F32, tag="low")
            for c in range(HC):
                nc.tensor.matmul(
                    out=low_ps,
                    lhsT=acat_sb[:, c, :],
                    rhs=xt_r[:, c, :],
                    start=(c == 0),
                    stop=(c == HC - 1),
                )
            st["low_ps"] = low_ps

        def emit_lg(i):
            st = st_by_tile[i]
            ht_sb = st["ht_sb"]
            lg = ps_lg.tile([128, JT, E], F32, tag="lg")
            st["lg"] = lg
            for q in range(JT):
                for c2 in range(RC):
                    nc.tensor.matmul(
                        out=lg[:, q, :],
                        lhsT=ht_sb[:, c2, ts(q, 128)],
                        rhs=w2_sb[:, c2, :],
                        start=(c2 == 0),
                        stop=(c2 == RC - 1),
                    )

        def emit_topk_a(i):
            st = st_by_tile[i]
            lg = st["lg"]
            # bias fold: lgb = lg + b2 (b2 host-replicated across partitions);
            # downstream top-k ops then read SBUF, not PSUM (cheaper on DVE)
            lgb = small_p.tile([128, JT, E], F32, tag="lgb")
            nc.vector.tensor_tensor(out=lgb, in0=lg, in1=b2bc_sb, op=ALU.add)
            m1 = small_p.tile([128, JT], F32, tag="m1")
            nc.vector.tensor_reduce(out=m1, in_=lgb, axis=AX.X, op=ALU.max)
            top1 = small_p.tile([128, JT, E], F32, tag="top1")
            nc.vector.tensor_tensor(
                out=top1,
                in0=lgb,
                in1=m1.unsqueeze(-1).broadcast_to([128, JT, E]),
                op=ALU.is_equal,
            )
            masked = small_p.tile([128, JT, E], F32, tag="masked")
            nc.vector.scalar_tensor_tensor(
                out=masked, in0=top1, scalar=-1e30, in1=lgb,
                op0=ALU.mult, op1=ALU.add,
            )
            # SBUF-only ops on gpsimd (Pool)
            m2 = small_p.tile([128, JT], F32, tag="m2")
            nc.vector.tensor_reduce(out=m2, in_=masked, axis=AX.X, op=ALU.max)
            dlg = small_p.tile([128, JT], F32, tag="dlg")
            nc.gpsimd.tensor_tensor(out=dlg, in0=m2, in1=m1, op=ALU.subtract)
            # sigmoid(d) = 0.5 + 0.5*tanh(d/2); tanh shares the silu ACT
            # table set, so the steady loop never reloads the function table
            th = small_p.tile([128, JT], F32, tag="th")
            nc.scalar.activation(out=th, in_=dlg, func=AF.Tanh, scale=0.5)
            st["m1"], st["m2"], st["masked"], st["top1"], st["th"] = \
                m1, m2, masked, top1, th

        def emit_topk_b(i):
            st = st_by_tile[i]
            m1, m2, masked, top1, th = (st["m1"], st["m2"], st["masked"],
                                        st["top1"], st["th"])
            u2 = small_p.tile([128, JT], F32, tag="u2")
            nc.gpsimd.tensor_scalar(
                out=u2, in0=th, scalar1=0.5, scalar2=0.5,
                op0=ALU.mult, op1=ALU.add,
            )
            u1 = small_p.tile([128, JT], F32, tag="u1")
            nc.gpsimd.tensor_scalar(
                out=u1, in0=th, scalar1=-0.5, scalar2=0.5,
                op0=ALU.mult, op1=ALU.add,
            )
            top2 = small_p.tile([128, JT, E], F32, tag="top2")
            nc.vector.tensor_tensor(
                out=top2,
                in0=masked,
                in1=m2.unsqueeze(-1).broadcast_to([128, JT, E]),
                op=ALU.is_equal,
            )
            w_full = small_p.tile([128, JT, ER], FP16)
            wt2 = small_p.tile([128, JT, ER], F32, tag="wt2")
            w4 = w_full.rearrange("p j (e r) -> p j e r", r=R)
            wt24 = wt2.rearrange("p j (e r) -> p j e r", r=R)
            t14 = top1.unsqueeze(-1).broadcast_to([128, JT, E, R])
            t24 = top2.unsqueeze(-1).broadcast_to([128, JT, E, R])
            u14 = u1.unsqueeze(-1).unsqueeze(-1).broadcast_to([128, JT, E, R])
            u24 = u2.unsqueeze(-1).unsqueeze(-1).broadcast_to([128, JT, E, R])
            nc.vector.tensor_tensor(out=wt24, in0=t24, in1=u24, op=ALU.mult)
            nc.vector.tensor_tensor(out=w4, in0=t14, in1=u14, op=ALU.mult)
            nc.gpsimd.tensor_tensor(out=w_full, in0=w_full, in1=wt2, op=ALU.add)
            st["w_full"] = w_full

        def emit_wrt(i):
            st = st_by_tile[i]
            w_full = st["w_full"]
            wrt_ps = ps_wrt.tile([ER, JT, 128], FP16, tag="wrt")
            for q in range(JT):
                nc.tensor.transpose(
                    out=wrt_ps[:, q, :],
                    in_=w_full[:, q, :],
                    identity=ident_h,
                )
            wrt_sb = small_p.tile([ER, JT, 128], FP16, tag="wrt_sb")
            nc.vector.tensor_copy(out=wrt_sb, in_=wrt_ps)
            lw_sb = lw_p.tile([ER, TT], FP16)
            nc.vector.scalar_tensor_tensor(
                out=lw_sb,
                in0=st["low_ps"],
                scalar=float(SCALING),
                in1=wrt_sb.rearrange("p j t -> p (j t)"),
                op0=ALU.mult,
                op1=ALU.mult,
            )
            st["lw_sb"] = lw_sb

        def emit_delta_q(i, q):
            """q<2: PE preloads base into PSUM, delta accumulates, ACT Copy
            evacuates. q>=2: plain delta matmul, DVE add with base tile.
            Emitted per q, interleaved with router halves in PE program
            order so the bufs=1 dla/dlb WAR hides behind router streams."""
            st = st_by_tile[i]
            lw_r = st["lw_sb"]
            bs = st["bs"]
            if q == 0:
                bo = bout_p.tile([128, JT, H], OUT_DT)
                st["bo"] = bo
            bo = st["bo"]
            if True:
                dla = ps_dl.tile([128, 320], F32, tag="dla")
                dlb = ps_dl.tile([128, 320], F32, tag="dlb")
                pre = q < 2
                if pre:
                    nc.tensor.matmul(
                        out=dla, lhsT=ident_h, rhs=bs[:, q, 0:320],
                        start=True, stop=False,
                    )
                    nc.tensor.matmul(
                        out=dlb, lhsT=ident_h, rhs=bs[:, q, 320:H],
                        start=True, stop=False,
                    )
                nc.tensor.matmul(
                    out=dla, lhsT=lw_r[:, ts(q, 128)], rhs=bcat_sb[:, 0:320],
                    start=not pre, stop=True,
                )
                nc.tensor.matmul(
                    out=dlb, lhsT=lw_r[:, ts(q, 128)], rhs=bcat_sb[:, 320:H],
                    start=not pre, stop=True,
                )
                if pre:
                    nc.scalar.activation(
                        out=bo[:, q, 0:320], in_=dla, func=AF.Copy)
                    nc.scalar.activation(
                        out=bo[:, q, 320:H], in_=dlb, func=AF.Copy)
                else:
                    nc.vector.tensor_tensor(
                        out=bo[:, q, 0:320], in0=dla, in1=bs[:, q, 0:320],
                        op=ALU.add)
                    nc.vector.tensor_tensor(
                        out=bo[:, q, 320:H], in0=dlb, in1=bs[:, q, 320:H],
                        op=ALU.add)

        def emit_store(i):
            st = st_by_tile[i]
            nc.scalar.dma_start(
                out=out_d[i * TT : (i + 1) * TT, :].rearrange(
                    "(q p) h -> p q h", p=128
                ),
                in_=st["bo"],
            )

        xcs = {}
        if not comp_only:
            for g in range(min(2, nxc)):
                xcs[g] = emit_xchunk(g)

        nsteps = ntiles + 4
        for s in range(nsteps):
            g_need = (s + 2) // tiles_per_xc
            if not comp_only:
                for g in range(len(xcs), min(g_need + 1, nxc)):
                    xcs[g] = emit_xchunk(g)
            if s < ntiles:
                g, j = divmod(s, tiles_per_xc)
                st_by_tile[s] = {
                    "xt_sb": (xt_res if comp_only
                              else xcs[g][:, :, j * TT : (j + 1) * TT]),
                }
                if comp_only:
                    st_by_tile[s]["bs"] = zeros16
                else:
                    emit_bs(s)

            if dma_only:
                if s < ntiles:
                    st_by_tile[s]["bo"] = zeros_out
                    emit_store(s)
                continue

            if s < ntiles:
                emit_router_half(s, 0)
                emit_router_half(s, 1)
            if 0 <= s - 1 < ntiles:
                emit_lg(s - 1)
                emit_topk_a(s - 1)
            if 0 <= s - 2 < ntiles:
                emit_wrt(s - 2)
            if 0 <= s - 1 < ntiles:
                emit_low(s - 1)
                emit_topk_b(s - 1)
            if s < ntiles:
                emit_ht(s)
            if 0 <= s - 3 < ntiles:
                for q in range(JT):
                    emit_delta_q(s - 3, q)
            if not comp_only and 0 <= s - 4 < ntiles:
                emit_store(s - 4)

    return nc


_CACHE = {}


def _get_nc(t_core=T_CORE, niter=1, timing_mode=False, mode="full",
            fp16_out=FP16_OUT):
    key = (t_core, niter, timing_mode, mode, fp16_out)
    if key not in _CACHE:
        nc = build_kernel(t_core, niter, timing_mode, mode, fp16_out)
        nc.finalize()
        _CACHE[key] = nc
    return _CACHE[key]


def kernel(x, base_output, W1, b1, W2, b2, A, Bm):
    x = np.asarray(x)
    base_output = np.asarray(base_output)
    B, S, _ = x.shape
    assert B * S == N_CORES * T_CORE

    # host-side prep: shard per core, downcast to fp16, pre-transpose x,
    # pack all small constants into one [128, NCONST] blob (one DMA on device)
    xs = x.reshape(N_CORES, T_CORE, H)
    xT = np.ascontiguousarray(xs.transpose(0, 2, 1)).astype(np.float16)
    bb = base_output.reshape(N_CORES, T_CORE, H).astype(np.float16)
    W1h = np.asarray(W1, dtype=np.float16)
    W2h = np.asarray(W2, dtype=np.float16)
    Ah = np.asarray(A, dtype=np.float16)
    Bmh = np.asarray(Bm, dtype=np.float16)
    b2h = np.asarray(b2, dtype=np.float16)

    cb = np.zeros((128, NCONST), np.float16)
    cb[:, OFF_W1:OFF_W1 + HC * RH] = (
        W1h.reshape(HC, 128, RH).transpose(1, 0, 2).reshape(128, HC * RH))
    cb[:, OFF_A:OFF_A + HC * E * R] = (
        Ah.reshape(E, HC, 128, R).transpose(2, 1, 0, 3).reshape(128, HC * E * R))
    cb[:, OFF_W2:OFF_W2 + RC * E] = (
        W2h.reshape(RC, 128, E).transpose(1, 0, 2).reshape(128, RC * E))
    cb[0:ER, OFF_B:OFF_B + H] = Bmh.reshape(ER, H)
    cb[:, OFF_B2:OFF_B2 + JT * E] = np.tile(b2h, JT)[None, :]
    b1f = np.ascontiguousarray(
        np.asarray(b1, dtype=np.float32).reshape(RC, 128).T)

    nc = _get_nc()
    in_maps = [
        {
            "xT": xT[i],
            "bb": np.ascontiguousarray(bb[i]),
            "cb": cb, "b1": b1f,
        }
        for i in range(N_CORES)
    ]
    res = run_bass_kernel_spmd(nc, in_maps, list(range(N_CORES))).results
    out = np.stack([res[i]["out"] for i in range(N_CORES)], axis=0)
    return out.reshape(B, S, H).astype(np.float32)


# revision 6
# speedup vs baseline: 2.3173x; 1.0099x over previous
"""MoLoRA Trainium2 Bass kernel — r28.

Design (fp16 compute, rel-err ~5e-3 vs gate 2e-2):
- Host prep: shard per core, downcast to fp16, pre-transpose x to x^T [H,T]
  so the device does plain contiguous DMA loads (no transposes on device).
- Router SiLU fused into one ACT Silu activation (PSUM source).
- Router bias b2 folded into a host-replicated broadcast constant added on
  DVE (no bias matmuls; top-k reads SBUF instead of PSUM).
- Top-k small ops split: PSUM-readers on DVE, SBUF-only ops on gpsimd.
- Delta + base: for q<2 PE preloads base into PSUM (identity matmul) and the
  delta matmul accumulates, evacuated by ACT Copy; for q>=2 plain delta
  matmul evacuated by a DVE add with the base tile. Balances PE/ACT/DVE.
- Software-pipelined emission: per step s PE runs router(s), wrt(s-2),
  low(s-1)+logits(s-1), delta(s-3); store(s-4) on the ACT HWDGE ring.
- Output fp16 from device, upcast to f32 on host (fp16_out=True).
"""

import numpy as np
from contextlib import ExitStack

import concourse.bass as bass
import concourse.tile as tile
from concourse import bacc
from concourse import mybir
from concourse.bass import ts
from concourse.masks import make_identity
from concourse.bass_utils import run_bass_kernel_spmd

F32 = mybir.dt.float32
FP16 = mybir.dt.float16
AF = mybir.ActivationFunctionType
ALU = mybir.AluOpType
AX = mybir.AxisListType

H = 640
E = 5
R = 8
ER = E * R
RH = 256
HC = H // 128
RC = RH // 128
SCALING = 16.0 / R
N_CORES = 8
T_CORE = 4096
TT = 512          # compute tile
JT = TT // 128    # 4
XB = 1024         # x^T load-chunk tokens
FP16_OUT = True

# host-packed fp16 const blob layout (per partition)
OFF_W1 = 0                      # [HC, RH]     w1_sb[p, c, m] = W1[c*128+p, m]
OFF_A = OFF_W1 + HC * RH        # [HC, E, R]   acat[p, c, e, r] = A[e, c*128+p, r]
OFF_W2 = OFF_A + HC * E * R     # [RC, E]      w2[p, c2, e] = W2[c2*128+p, e]
OFF_B = OFF_W2 + RC * E         # rows 0..ER: bcat[er, h] = Bm[e, r, h]
OFF_B2 = OFF_B + H              # all rows: b2 tiled JT times (broadcast add)
NCONST = OFF_B2 + JT * E


def build_kernel(t_core=T_CORE, niter=1, timing_mode=False, mode="full",
                 fp16_out=FP16_OUT):
    ntiles = t_core // TT
    tiles_per_xc = XB // TT
    nxc = t_core // XB
    OUT_DT = FP16 if fp16_out else F32
    nc = bacc.Bacc()

    if timing_mode:
        xT_d = nc.dram_tensor("xT_int", [H, t_core], FP16)[:, :]
        bb_d = nc.dram_tensor("bb_int", [t_core, H], FP16)[:, :]
        out_d = nc.dram_tensor("out_int", [t_core, H], OUT_DT)[:, :]
        dummy_d = nc.declare_dram_parameter("dummy_out", [1, 4], F32, isOutput=True)
        cb_d = nc.dram_tensor("cb_int", [128, NCONST], FP16)[:, :]
        b1_d = nc.dram_tensor("b1_int", [128, RC], F32)[:, :]
    else:
        xT_d = nc.declare_dram_parameter("xT", [H, t_core], FP16, isOutput=False)[:, :]
        bb_d = nc.declare_dram_parameter("bb", [t_core, H], FP16, isOutput=False)[:, :]
        out_d = nc.declare_dram_parameter("out", [t_core, H], OUT_DT, isOutput=True)[:, :]
        dummy_d = None
        cb_d = nc.declare_dram_parameter("cb", [128, NCONST], FP16, isOutput=False)[:, :]
        b1_d = nc.declare_dram_parameter("b1", [128, RC], F32, isOutput=False)[:, :]

    with ExitStack() as ctx:
        tc = ctx.enter_context(tile.TileContext(nc))
        const = ctx.enter_context(tc.tile_pool(name="const", bufs=1))
        bs_p = ctx.enter_context(tc.tile_pool(name="bs", bufs=6))
        bout_p = ctx.enter_context(tc.tile_pool(name="bout", bufs=5))
        xt_p = ctx.enter_context(tc.tile_pool(name="xt", bufs=3))
        ht_p = ctx.enter_context(tc.tile_pool(name="ht", bufs=3))
        small_p = ctx.enter_context(tc.tile_pool(name="small", bufs=6))
        lw_p = ctx.enter_context(tc.tile_pool(name="lw", bufs=4))
        ps_rt = ctx.enter_context(tc.tile_pool(name="ps_rt", bufs=1, space="PSUM"))
        ps_lg = ctx.enter_context(tc.tile_pool(name="ps_lg", bufs=1, space="PSUM"))
        ps_low = ctx.enter_context(tc.tile_pool(name="ps_low", bufs=2, space="PSUM"))
        ps_wrt = ctx.enter_context(tc.tile_pool(name="ps_wrt", bufs=1, space="PSUM"))
        ps_dl = ctx.enter_context(tc.tile_pool(name="ps_dl", bufs=1, space="PSUM"))

        ident = const.tile([128, 128], F32)
        make_identity(nc, ident)
        ident_h = const.tile([128, 128], FP16)
        nc.vector.tensor_copy(out=ident_h, in_=ident)

        # all fp16 constants arrive host-packed in one blob -> one fast DMA
        cb = const.tile([128, NCONST], FP16)
        nc.sync.dma_start(out=cb, in_=cb_d)
        b1_sb = const.tile([128, RC], F32)
        nc.sync.dma_start(out=b1_sb, in_=b1_d)
        w1_sb = cb[:, OFF_W1 : OFF_W1 + HC * RH].rearrange("p (c m) -> p c m", c=HC)
        acat_sb = cb[:, OFF_A : OFF_A + HC * E * R].rearrange(
            "p (c d) -> p c d", c=HC)
        w2_sb = cb[:, OFF_W2 : OFF_W2 + RC * E].rearrange("p (c e) -> p c e", c=RC)
        bcat_sb = cb[0:ER, OFF_B : OFF_B + H]
        b2bc_sb = cb[:, OFF_B2 : OFF_B2 + JT * E].rearrange(
            "p (q e) -> p q e", q=JT)

        if mode != "full":
            zeros16 = const.tile([128, JT, H], FP16)
            nc.vector.memset(zeros16, 0.0)
            zeros_out = const.tile([128, JT, H], OUT_DT)
            nc.vector.memset(zeros_out, 0.0)

        xt_res = None
        if mode == "comp":
            xt_res = const.tile([128, HC, TT], FP16)
            nc.sync.dma_start(
                out=xt_res,
                in_=xT_d[:, 0:TT].rearrange("(c p) t -> p c t", p=128),
            )

        if dummy_d is not None:
            dnm = const.tile([1, 4], F32)
            nc.vector.memset(dnm, 1.0)
            nc.sync.dma_start(out=dummy_d[:, :], in_=dnm)

        loop_ctx = tc.For_i(0, niter, 1) if niter > 1 else None
        if loop_ctx is not None:
            ctx.enter_context(loop_ctx)

        dma_only = mode == "dma"
        comp_only = mode == "comp"
        st_by_tile = {}

        def emit_xchunk(g):
            xt_sb = xt_p.tile([128, HC, XB], FP16)
            nc.sync.dma_start(
                out=xt_sb,
                in_=xT_d[:, g * XB : (g + 1) * XB].rearrange(
                    "(c p) t -> p c t", p=128
                ),
            )
            return xt_sb

        def emit_bs(i):
            bs = bs_p.tile([128, JT, H], FP16)
            nc.sync.dma_start(
                out=bs,
                in_=bb_d[i * TT : (i + 1) * TT, :].rearrange(
                    "(q p) h -> p q h", p=128
                ),
            )
            st_by_tile[i]["bs"] = bs

        def emit_router_half(i, c2):
            st = st_by_tile[i]
            xt_r = st["xt_sb"]
            if c2 == 0:
                h_ps = ps_rt.tile([128, RC, TT], F32, tag="rt")
                st["h_ps"] = h_ps
            h_ps = st["h_ps"]
            for c in range(HC):
                nc.tensor.matmul(
                    out=h_ps[:, c2, :],
                    lhsT=w1_sb[:, c, ts(c2, 128)],
                    rhs=xt_r[:, c, :],
                    start=(c == 0),
                    stop=(c == HC - 1),
                )

        def emit_ht(i):
            st = st_by_tile[i]
            h_ps = st["h_ps"]
            ht_sb = ht_p.tile([128, RC, TT], FP16)
            for c2 in range(RC):
                nc.scalar.activation(
                    out=ht_sb[:, c2, :],
                    in_=h_ps[:, c2, :],
                    func=AF.Silu,
                    bias=b1_sb[:, c2 : c2 + 1],
                )
            st["ht_sb"] = ht_sb

        def emit_low(i):
            st = st_by_tile[i]
            xt_r = st["xt_sb"]
            low_ps = ps_low.tile([ER, TT], F32, tag="low")
            for c in range(HC):
                nc.tensor.matmul(
                    out=low_ps,
                    lhsT=acat_sb[:, c, :],
                    rhs=xt_r[:, c, :],
                    start=(c == 0),
                    stop=(c == HC - 1),
                )
            st["low_ps"] = low_ps

        def emit_lg(i):
            st = st_by_tile[i]
            ht_sb = st["ht_sb"]
            lg = ps_lg.tile([128, JT, E], F32, tag="lg")
            st["lg"] = lg
            for q in range(JT):
                for c2 in range(RC):
                    nc.tensor.matmul(
                        out=lg[:, q, :],
                        lhsT=ht_sb[:, c2, ts(q, 128)],
                        rhs=w2_sb[:, c2, :],
                        start=(c2 == 0),
                        stop=(c2 == RC - 1),
                    )

        def emit_topk_a(i):
            st = st_by_tile[i]
            lg = st["lg"]
            # bias fold: lgb = lg + b2 (b2 host-replicated across partitions);
            # downstream top-k ops then read SBUF, not PSUM (cheaper on DVE)
            lgb = small_p.tile([128, JT, E], F32, tag="lgb")
            nc.vector.tensor_tensor(out=lgb, in0=lg, in1=b2bc_sb, op=ALU.add)
            m1 = small_p.tile([128, JT], F32, tag="m1")
            nc.vector.tensor_reduce(out=m1, in_=lgb, axis=AX.X, op=ALU.max)
            top1 = small_p.tile([128, JT, E], F32, tag="top1")
            nc.vector.tensor_tensor(
                out=top1,
                in0=lgb,
                in1=m1.unsqueeze(-1).broadcast_to([128, JT, E]),
                op=ALU.is_equal,
            )
            masked = small_p.tile([128, JT, E], F32, tag="masked")
            nc.vector.scalar_tensor_tensor(
                out=masked, in0=top1, scalar=-1e30, in1=lgb,
                op0=ALU.mult, op1=ALU.add,
            )
            # SBUF-only ops on gpsimd (Pool)
            m2 = small_p.tile([128, JT], F32, tag="m2")
            nc.vector.tensor_reduce(out=m2, in_=masked, axis=AX.X, op=ALU.max)
            dlg = small_p.tile([128, JT], F32, tag="dlg")
            nc.gpsimd.tensor_tensor(out=dlg, in0=m2, in1=m1, op=ALU.subtract)
            # sigmoid(d) = 0.5 + 0.5*tanh(d/2); tanh shares the silu ACT
            # table set, so the steady loop never reloads the function table
            th = small_p.tile([128, JT], F32, tag="th")
            nc.scalar.activation(out=th, in_=dlg, func=AF.Tanh, scale=0.5)
            st["m1"], st["m2"], st["masked"], st["top1"], st["th"] = \
                m1, m2, masked, top1, th

        def emit_topk_b(i):
            st = st_by_tile[i]
            m1, m2, masked, top1, th = (st["m1"], st["m2"], st["masked"],
                                        st["top1"], st["th"])
            u2 = small_p.tile([128, JT], F32, tag="u2")
            nc.gpsimd.tensor_scalar(
                out=u2, in0=th, scalar1=0.5, scalar2=0.5,
                op0=ALU.mult, op1=ALU.add,
            )
            u1 = small_p.tile([128, JT], F32, tag="u1")
            nc.gpsimd.tensor_scalar(
                out=u1, in0=th, scalar1=-0.5, scalar2=0.5,
                op0=ALU.mult, op1=ALU.add,
            )
            top2 = small_p.tile([128, JT, E], F32, tag="top2")
            nc.vector.tensor_tensor(
                out=top2,
                in0=masked,
                in1=m2.unsqueeze(-1).broadcast_to([128, JT, E]),
                op=ALU.is_equal,
            )
            w_full = small_p.tile([128, JT, ER], FP16)
            wt2 = small_p.tile([128, JT, ER], F32, tag="wt2")
            w4 = w_full.rearrange("p j (e r) -> p j e r", r=R)
            wt24 = wt2.rearrange("p j (e r) -> p j e r", r=R)
            t14 = top1.unsqueeze(-1).broadcast_to([128, JT, E, R])
            t24 = top2.unsqueeze(-1).broadcast_to([128, JT, E, R])
            u14 = u1.unsqueeze(-1).unsqueeze(-1).broadcast_to([128, JT, E, R])
            u24 = u2.unsqueeze(-1).unsqueeze(-1).broadcast_to([128, JT, E, R])
            nc.vector.tensor_tensor(out=wt24, in0=t24, in1=u24, op=ALU.mult)
            nc.vector.tensor_tensor(out=w4, in0=t14, in1=u14, op=ALU.mult)
            nc.gpsimd.tensor_tensor(out=w_full, in0=w_full, in1=wt2, op=ALU.add)
            st["w_full"] = w_full

        def emit_wrt(i):
            st = st_by_tile[i]
            w_full = st["w_full"]
            wrt_ps = ps_wrt.tile([ER, JT, 128], FP16, tag="wrt")
            for q in range(JT):
                nc.tensor.transpose(
                    out=wrt_ps[:, q, :],
                    in_=w_full[:, q, :],
                    identity=ident_h,
                )
            wrt_sb = small_p.tile([ER, JT, 128], FP16, tag="wrt_sb")
            nc.vector.tensor_copy(out=wrt_sb, in_=wrt_ps)
            lw_sb = lw_p.tile([ER, TT], FP16)
            nc.vector.scalar_tensor_tensor(
                out=lw_sb,
                in0=st["low_ps"],
                scalar=float(SCALING),
                in1=wrt_sb.rearrange("p j t -> p (j t)"),
                op0=ALU.mult,
                op1=ALU.mult,
            )
            st["lw_sb"] = lw_sb

        def emit_delta_q(i, q):
            """q<2: PE preloads base into PSUM, delta accumulates, ACT Copy
            evacuates. q>=2: plain delta matmul, DVE add with base tile.
            Emitted per q, interleaved with router halves in PE program
            order so the bufs=1 dla/dlb WAR hides behind router streams."""
            st = st_by_tile[i]
            lw_r = st["lw_sb"]
            bs = st["bs"]
            if q == 0:
                bo = bout_p.tile([128, JT, H], OUT_DT)
                st["bo"] = bo
            bo = st["bo"]
            if True:
                dla = ps_dl.tile([128, 320], F32, tag="dla")
                dlb = ps_dl.tile([128, 320], F32, tag="dlb")
                pre = q < 2
                if pre:
                    nc.tensor.matmul(
                        out=dla, lhsT=ident_h, rhs=bs[:, q, 0:320],
                        start=True, stop=False,
                    )
                    nc.tensor.matmul(
                        out=dlb, lhsT=ident_h, rhs=bs[:, q, 320:H],
                        start=True, stop=False,
                    )
                nc.tensor.matmul(
                    out=dla, lhsT=lw_r[:, ts(q, 128)], rhs=bcat_sb[:, 0:320],
                    start=not pre, stop=True,
                )
                nc.tensor.matmul(
                    out=dlb, lhsT=lw_r[:, ts(q, 128)], rhs=bcat_sb[:, 320:H],
                    start=not pre, stop=True,
                )
                if pre:
                    nc.scalar.activation(
                        out=bo[:, q, 0:320], in_=dla, func=AF.Copy)
                    nc.scalar.activation(
                        out=bo[:, q, 320:H], in_=dlb, func=AF.Copy)
                else:
                    nc.vector.tensor_tensor(
                        out=bo[:, q, 0:320], in0=dla, in1=bs[:, q, 0:320],
                        op=ALU.add)
                    nc.vector.tensor_tensor(
                        out=bo[:, q, 320:H], in0=dlb, in1=bs[:, q, 320:H],
                        op=ALU.add)

        def emit_store(i):
            st = st_by_tile[i]
            nc.scalar.dma_start(
                out=out_d[i * TT : (i + 1) * TT, :].rearrange(
                    "(q p) h -> p q h", p=128
                ),
                in_=st["bo"],
            )

        xcs = {}
        if not comp_only:
            for g in range(min(2, nxc)):
                xcs[g] = emit_xchunk(g)

        nsteps = ntiles + 4
        for s in range(nsteps):
            g_need = (s + 2) // tiles_per_xc
            if not comp_only:
                for g in range(len(xcs), min(g_need + 1, nxc)):
                    xcs[g] = emit_xchunk(g)
            if s < ntiles:
                g, j = divmod(s, tiles_per_xc)
                st_by_tile[s] = {
                    "xt_sb": (xt_res if comp_only
                              else xcs[g][:, :, j * TT : (j + 1) * TT]),
                }
                if comp_only:
                    st_by_tile[s]["bs"] = zeros16
                else:
                    emit_bs(s)

            if dma_only:
                if s < ntiles:
                    st_by_tile[s]["bo"] = zeros_out
                    emit_store(s)
                continue

            if s < ntiles:
                emit_router_half(s, 0)
                emit_router_half(s, 1)
            if 0 <= s - 1 < ntiles:
                emit_lg(s - 1)
                emit_topk_a(s - 1)
            if 0 <= s - 2 < ntiles:
                emit_wrt(s - 2)
            if 0 <= s - 1 < ntiles:
                emit_low(s - 1)
                emit_topk_b(s - 1)
            if s < ntiles:
                emit_ht(s)
            if 0 <= s - 3 < ntiles:
                for q in range(JT):
                    emit_delta_q(s - 3, q)
            if not comp_only and 0 <= s - 4 < ntiles:
                emit_store(s - 4)

    return nc


_CACHE = {}


def _get_nc(t_core=T_CORE, niter=1, timing_mode=False, mode="full",
            fp16_out=FP16_OUT):
    key = (t_core, niter, timing_mode, mode, fp16_out)
    if key not in _CACHE:
        nc = build_kernel(t_core, niter, timing_mode, mode, fp16_out)
        nc.finalize()
        _CACHE[key] = nc
    return _CACHE[key]


def kernel(x, base_output, W1, b1, W2, b2, A, Bm):
    x = np.asarray(x)
    base_output = np.asarray(base_output)
    B, S, _ = x.shape
    assert B * S == N_CORES * T_CORE

    # host-side prep: shard per core, downcast to fp16, pre-transpose x,
    # pack all small constants into one [128, NCONST] blob (one DMA on device)
    xs = x.reshape(N_CORES, T_CORE, H)
    xT = np.ascontiguousarray(xs.transpose(0, 2, 1)).astype(np.float16)
    bb = base_output.reshape(N_CORES, T_CORE, H).astype(np.float16)
    W1h = np.asarray(W1, dtype=np.float16)
    W2h = np.asarray(W2, dtype=np.float16)
    Ah = np.asarray(A, dtype=np.float16)
    Bmh = np.asarray(Bm, dtype=np.float16)
    b2h = np.asarray(b2, dtype=np.float16)

    cb = np.zeros((128, NCONST), np.float16)
    cb[:, OFF_W1:OFF_W1 + HC * RH] = (
        W1h.reshape(HC, 128, RH).transpose(1, 0, 2).reshape(128, HC * RH))
    cb[:, OFF_A:OFF_A + HC * E * R] = (
        Ah.reshape(E, HC, 128, R).transpose(2, 1, 0, 3).reshape(128, HC * E * R))
    cb[:, OFF_W2:OFF_W2 + RC * E] = (
        W2h.reshape(RC, 128, E).transpose(1, 0, 2).reshape(128, RC * E))
    cb[0:ER, OFF_B:OFF_B + H] = Bmh.reshape(ER, H)
    cb[:, OFF_B2:OFF_B2 + JT * E] = np.tile(b2h, JT)[None, :]
    b1f = np.ascontiguousarray(
        np.asarray(b1, dtype=np.float32).reshape(RC, 128).T)

    nc = _get_nc()
    in_maps = [
        {
            "xT": xT[i],
            "bb": np.ascontiguousarray(bb[i]),
            "cb": cb, "b1": b1f,
        }
        for i in range(N_CORES)
    ]
    res = run_bass_kernel_spmd(nc, in_maps, list(range(N_CORES))).results
    out = np.stack([res[i]["out"] for i in range(N_CORES)], axis=0)
    return out.reshape(B, S, H).astype(np.float32)
